# revision 10
# baseline (speedup 1.0000x reference)
"""Chamfer-based independent 3D pose adversarial loss on 8 Trainium2 cores.

Math (matches the jax reference exactly, using monotonicity of arccos/sqrt/clip):
  tr[r,f]   = <R_fake[f], r_buffer[r]>            (frobenius, K=9 matmul)
  nsq[r,f]  = -(|tb_r|^2 + |tf_f|^2 - 2 tb_r.tf_f) (K=5 matmul, negated -> all max)
  rd2 = mean_r arccos(clip(0.5(max_f tr - 1)))    (needs cross-core max over f shards)
  rd1 = mean_f arccos(clip(0.5(max_r tr - 1)))    (local; cross-core add of partial sums)
  td1 = mean_r sqrt(relu(-max_f nsq) + eps)       (cross-core max)
  td2 = mean_f sqrt(relu(-max_r nsq) + eps)       (local; cross-core add)
  out = rd1 + rd2 + pi * (td1 + td2)

Sharding: nF (4096) split across 8 cores, 512 fake poses per core; nR full.
Matmuls run in bf16 hi/lo split form (3 cross products, error ~1e-5 relative).
arccos(x) = 2*atan(sqrt((1-x)/(1+x))), with atan(t>1) = pi/2 - atan(1/t).
"""

import os
import sys

import numpy as np

for _p in ("/opt/trn_rl_repo",):
    if _p not in sys.path and os.path.isdir(_p):
        sys.path.insert(0, _p)

import ml_dtypes  # noqa: E402

import concourse.bacc as bacc  # noqa: E402
import concourse.mybir as mybir  # noqa: E402
from concourse import tile  # noqa: E402
from concourse.bass_utils import run_bass_kernel_spmd  # noqa: E402

NR = 8192
NF = 4096
NCORES = 8
NFL = NF // NCORES          # 512 fake poses per core
RB = NR // 128              # 64 r-blocks of 128
K_ROT = 27                  # 9 * 3 split terms (hh, hl, lh)
K_SQ = 15                   # 5 * 3
EVAC_DT = mybir.dt.float16
NEG_BIG = -60000.0
EPS_CH = 1e-5
EPS_ACOS = 1e-6
TR_LO = -1.0 + 2.0 * EPS_ACOS   # clip on tr equivalent to clip on cos=(tr-1)/2
TR_HI = 3.0 - 2.0 * EPS_ACOS
W = RB + 4                  # packed work width: 64 r-side cols + 4 f-side cols

_F32 = mybir.dt.float32
_BF16 = mybir.dt.bfloat16

_CACHE = {}


def _bf16_split(x):
    """x (fp32) -> (hi, lo) bf16 with hi + lo ~= x."""
    bf = ml_dtypes.bfloat16
    hi = x.astype(bf)
    lo = (x - hi.astype(np.float32)).astype(bf)
    return hi, lo


def _split_stack_left(xT):
    """Left operand stack [hi; hi; lo] for products hh + hl + lh."""
    hi, lo = _bf16_split(xT)
    return np.concatenate([hi, hi, lo], axis=0)


def _split_stack_right(xT):
    """Right operand stack [hi; lo; hi]."""
    hi, lo = _bf16_split(xT)
    return np.concatenate([hi, lo, hi], axis=0)


def _build_nc():
    nc = bacc.Bacc("TRN2", num_devices=NCORES, debug=False)
    mm = mybir.AluOpType

    lhsT_rot_d = nc.dram_tensor("lhsT_rot", [K_ROT, NR], _BF16, kind="ExternalInput")
    lhsT_sq_d = nc.dram_tensor("lhsT_sq", [K_SQ, NR], _BF16, kind="ExternalInput")
    rhs_rot_d = nc.dram_tensor("rhs_rot", [K_ROT, NFL], _BF16, kind="ExternalInput")
    rhs_sq_d = nc.dram_tensor("rhs_sq", [K_SQ, NFL], _BF16, kind="ExternalInput")
    ident_d = nc.dram_tensor("ident", [128, 128], EVAC_DT, kind="ExternalInput")
    out_d = nc.dram_tensor("out", [1, 1], _F32, kind="ExternalOutput")

    groups = [list(range(NCORES))]

    with tile.TileContext(nc) as tc:
        with (
            tc.tile_pool(name="const", bufs=1) as cpool,
            tc.tile_pool(name="mm", bufs=2, space="PSUM") as mmpool,
            tc.tile_pool(name="fin", bufs=1, space="PSUM") as finpool,
            tc.tile_pool(name="pair", bufs=4) as pairpool,
            tc.tile_pool(name="small", bufs=1) as spool,
            tc.tile_pool(name="dram", bufs=1, space="DRAM") as dpool,
        ):
            lhsT_rot = cpool.tile([K_ROT, NR], _BF16)
            lhsT_sq = cpool.tile([K_SQ, NR], _BF16)
            rhs_rot = cpool.tile([K_ROT, NFL], _BF16)
            rhs_sq = cpool.tile([K_SQ, NFL], _BF16)
            ident = cpool.tile([128, 128], EVAC_DT)
            nc.sync.dma_start(lhsT_rot[:], lhsT_rot_d[:])
            nc.sync.dma_start(lhsT_sq[:], lhsT_sq_d[:])
            nc.sync.dma_start(rhs_rot[:], rhs_rot_d[:])
            nc.sync.dma_start(rhs_sq[:], rhs_sq_d[:])
            nc.sync.dma_start(ident[:], ident_d[:])

            # Running per-f max over r-blocks; cols 0:512 rot tr, 512:1024 neg sq.
            acc = cpool.tile([128, 2 * NFL], EVAC_DT)
            nc.vector.memset(acc[:], NEG_BIG)
            # Per-r max over local f: [p, rb, 0] rot / [p, rb, 1] negsq, r = rb*128+p.
            # fp16 + innermost step-1 output keeps the DVE reduce in packed mode.
            maxf = cpool.tile([128, RB, 2], EVAC_DT)

            # ---- main loop over r-blocks ----
            for rb in range(RB):
                ps = mmpool.tile([128, 2 * NFL], _F32)
                cols = slice(rb * 128, (rb + 1) * 128)
                nc.tensor.matmul(
                    ps[:, 0:NFL], lhsT_rot[:, cols], rhs_rot[:], start=True, stop=True
                )
                nc.tensor.matmul(
                    ps[:, NFL : 2 * NFL], lhsT_sq[:, cols], rhs_sq[:],
                    start=True, stop=True,
                )
                pair = pairpool.tile([128, 2 * NFL], EVAC_DT)
                nc.scalar.copy(pair[:], ps[:])
                nc.vector.tensor_reduce(
                    maxf[:, rb, :],
                    pair[:].rearrange("p (t n) -> p t n", t=2),
                    axis=mybir.AxisListType.X,
                    op=mm.max,
                )
                nc.vector.tensor_tensor(acc[:], pair[:], acc[:], op=mm.max)

            # ---- per-f stats: transpose acc, reduce over the 128 r-partials ----
            ft = finpool.tile([128, 1024], EVAC_DT)
            ff = spool.tile([128, 2, 4], _F32)
            for t in range(8):
                nc.tensor.transpose(
                    ft[:, t * 128 : (t + 1) * 128],
                    acc[:, t * 128 : (t + 1) * 128],
                    ident[:],
                )
                nc.vector.tensor_reduce(
                    ff[:, t // 4, t % 4 : t % 4 + 1],
                    ft[:, t * 128 : (t + 1) * 128],
                    axis=mybir.AxisListType.X,
                    op=mm.max,
                )

            # ---- collective 1: elementwise max of per-r stats across cores ----
            cc1_in = dpool.tile([2, NR], EVAC_DT)
            cc1_out = dpool.tile([2, NR], EVAC_DT)
            nc.sync.dma_start(
                cc1_in[0, :].rearrange("(p n) -> p n", p=128), maxf[:, :, 0]
            )
            nc.sync.dma_start(
                cc1_in[1, :].rearrange("(p n) -> p n", p=128), maxf[:, :, 1]
            )
            nc.gpsimd.collective_compute(
                "AllReduce",
                mm.max,
                replica_groups=groups,
                ins=[cc1_in[:].opt()],
                outs=[cc1_out[:].opt()],
            )
            gstat = spool.tile([128, 2, RB], EVAC_DT)
            nc.sync.dma_start(
                gstat[:, 0, :], cc1_out[0, :].rearrange("(p n) -> p n", p=128)
            )
            nc.sync.dma_start(
                gstat[:, 1, :], cc1_out[1, :].rearrange("(p n) -> p n", p=128)
            )

            # ---- pack rot / negsq sources: cols 0:64 r-side, 64:68 f-side ----
            rot_src = spool.tile([128, W], _F32)
            neg_src = spool.tile([128, W], _F32)
            nc.vector.tensor_copy(rot_src[:, 0:RB], gstat[:, 0, :])
            nc.vector.tensor_copy(rot_src[:, RB:W], ff[:, 0, :])
            nc.vector.tensor_copy(neg_src[:, 0:RB], gstat[:, 1, :])
            nc.vector.tensor_copy(neg_src[:, RB:W], ff[:, 1, :])

            # ---- arccos pipeline on rot_src ----
            xc = spool.tile([128, W], _F32)
            num = spool.tile([128, W], _F32)
            den = spool.tile([128, W], _F32)
            ratio = spool.tile([128, W], _F32)
            root = spool.tile([128, W], _F32)
            rroot = spool.tile([128, W], _F32)
            mt = spool.tile([128, W], _F32)
            at = spool.tile([128, W], _F32)
            flag = spool.tile([128, W], _F32)
            sgn = spool.tile([128, W], _F32)
            ang2 = spool.tile([128, W], _F32)
            angle = spool.tile([128, W], _F32)
            zt = spool.tile([128, W], _F32)
            st = spool.tile([128, W], _F32)

            eps_ap = spool.tile([128, 1], _F32)
            nc.vector.memset(eps_ap[:], EPS_CH)

            nc.vector.tensor_scalar(xc[:], rot_src[:], TR_HI, TR_LO, mm.min, mm.max)
            nc.vector.tensor_scalar(num[:], xc[:], -0.5, 1.5, mm.mult, mm.add)
            nc.vector.tensor_scalar(den[:], xc[:], 0.5, 0.5, mm.mult, mm.add)
            nc.vector.reciprocal(den[:], den[:])
            nc.vector.tensor_tensor(ratio[:], num[:], den[:], op=mm.mult)
            # clamp of negsq for the sqrt path
            nc.vector.tensor_scalar_min(zt[:], neg_src[:], 0.0)

            # ACT phase 1: sqrt set (2 instructions)
            nc.scalar.activation(
                root[:], ratio[:], mybir.ActivationFunctionType.Sqrt
            )
            nc.scalar.activation(
                st[:], zt[:], mybir.ActivationFunctionType.Sqrt,
                bias=eps_ap[:], scale=-1.0,
            )

            nc.vector.reciprocal(rroot[:], root[:])
            nc.vector.tensor_tensor(mt[:], root[:], rroot[:], op=mm.min)

            # ACT phase 2: arctan set (1 instruction)
            nc.scalar.activation(at[:], mt[:], mybir.ActivationFunctionType.Arctan)

            # angle = 2*atan(root) = 2*at if root<=1 else pi - 2*at
            nc.vector.tensor_scalar(flag[:], root[:], 1.0, None, mm.is_gt)
            nc.vector.tensor_scalar(sgn[:], flag[:], -4.0, 2.0, mm.mult, mm.add)
            nc.vector.tensor_tensor(ang2[:], at[:], sgn[:], op=mm.mult)
            nc.vector.scalar_tensor_tensor(
                angle[:], flag[:], float(np.pi), ang2[:], op0=mm.mult, op1=mm.add
            )

            # ---- reduce to the four sums ----
            sums4 = spool.tile([128, 4], _F32)
            nc.vector.tensor_reduce(
                sums4[:, 0:1], angle[:, RB:W], axis=mybir.AxisListType.X, op=mm.add
            )  # rd1 partial (local f)
            nc.vector.tensor_reduce(
                sums4[:, 1:2], st[:, RB:W], axis=mybir.AxisListType.X, op=mm.add
            )  # td2 partial (local f)
            nc.vector.tensor_reduce(
                sums4[:, 2:3], angle[:, 0:RB], axis=mybir.AxisListType.X, op=mm.add
            )  # rd2 (global r)
            nc.vector.tensor_reduce(
                sums4[:, 3:4], st[:, 0:RB], axis=mybir.AxisListType.X, op=mm.add
            )  # td1 (global r)

            ones = spool.tile([128, 1], _F32)
            nc.vector.memset(ones[:], 1.0)
            s4 = finpool.tile([1, 4], _F32)
            nc.tensor.matmul(s4[:], ones[:], sums4[:], start=True, stop=True)

            # ---- collective 2: add the two f-side partial sums across cores ----
            z64 = spool.tile([1, 64], _F32)
            nc.vector.memset(z64[:], 0.0)
            nc.scalar.copy(z64[:, 0:2], s4[:, 0:2])
            cc2_in = dpool.tile([1, 64], _F32)
            cc2_out = dpool.tile([1, 64], _F32)
            nc.sync.dma_start(cc2_in[:], z64[:])
            nc.gpsimd.collective_compute(
                "AllReduce",
                mm.add,
                replica_groups=groups,
                ins=[cc2_in[:].opt()],
                outs=[cc2_out[:].opt()],
            )
            g2 = spool.tile([1, 2], _F32)
            nc.sync.dma_start(g2[:], cc2_out[:, 0:2])

            # ---- final combine ----
            t1 = spool.tile([1, 2], _F32)
            t2 = spool.tile([1, 2], _F32)
            ab = spool.tile([1, 2], _F32)
            res = spool.tile([1, 1], _F32)
            nc.vector.tensor_scalar_mul(t1[:], g2[:], 1.0 / NF)
            nc.vector.tensor_scalar_mul(t2[:], s4[:, 2:4], 1.0 / NR)
            nc.vector.tensor_tensor(ab[:], t1[:], t2[:], op=mm.add)
            nc.vector.scalar_tensor_tensor(
                res[:], ab[:, 1:2], float(np.pi), ab[:, 0:1],
                op0=mm.mult, op1=mm.add,
            )
            nc.sync.dma_start(out_d[:], res[:])

    nc.compile()
    return nc


def _prep_inputs(t_fake, R_fake, r_buffer, t_buffer):
    """Host-side shard + bf16 split. Returns per-core input maps."""
    t_fake = np.asarray(t_fake, dtype=np.float32)
    R_fake = np.asarray(R_fake, dtype=np.float32)
    r_buffer = np.asarray(r_buffer, dtype=np.float32)
    tb = np.asarray(t_buffer, dtype=np.float32)[0]          # [NR, 3]

    rotT = r_buffer.reshape(NR, 9).T                         # [9, NR]
    a_r = np.sum(tb * tb, axis=1)                            # [NR]
    # negated augmented vector: nsq = u_neg . v  with v = [1, |tf|^2, tf]
    u_negT = np.concatenate(
        [-a_r[None, :], -np.ones((1, NR), np.float32), 2.0 * tb.T], axis=0
    )                                                        # [5, NR]
    lhsT_rot = _split_stack_left(rotT)                       # [27, NR]
    lhsT_sq = _split_stack_left(u_negT)                      # [15, NR]

    ident = np.eye(128, dtype=np.float16)

    in_maps = []
    for c in range(NCORES):
        sl = slice(c * NFL, (c + 1) * NFL)
        RfT = R_fake.reshape(NF, 9)[sl].T                    # [9, NFL]
        tf = t_fake[sl]                                      # [NFL, 3]
        b_f = np.sum(tf * tf, axis=1)
        vT = np.concatenate(
            [np.ones((1, NFL), np.float32), b_f[None, :], tf.T], axis=0
        )                                                    # [5, NFL]
        in_maps.append(
            {
                "lhsT_rot": lhsT_rot,
                "lhsT_sq": lhsT_sq,
                "rhs_rot": _split_stack_right(RfT),          # [27, NFL]
                "rhs_sq": _split_stack_right(vT),            # [15, NFL]
                "ident": ident,
            }
        )
    return in_maps


LAST_RESULTS = None


def kernel(t_fake, R_fake, r_buffer, t_buffer):
    global LAST_RESULTS
    if "nc" not in _CACHE:
        _CACHE["nc"] = _build_nc()
    nc = _CACHE["nc"]
    in_maps = _prep_inputs(t_fake, R_fake, r_buffer, t_buffer)
    res = run_bass_kernel_spmd(nc, in_maps, list(range(NCORES)))
    LAST_RESULTS = res
    return np.float32(res.results[0]["out"][0, 0])


# revision 14
# speedup vs baseline: 2.3790x; 2.3790x over previous
"""Chamfer-based independent 3D pose adversarial loss on 8 Trainium2 cores.

Math (matches the jax reference exactly, using monotonicity of arccos/sqrt/clip):
  tr[r,f]   = <R_fake[f], r_buffer[r]>            (frobenius, K=9 matmul)
  nsq[r,f]  = -(|tb_r|^2 + |tf_f|^2 - 2 tb_r.tf_f) (K=5 matmul, negated -> all max)
  rd2 = mean_r arccos(clip(0.5(max_f tr - 1)))    (needs cross-core max over f shards)
  rd1 = mean_f arccos(clip(0.5(max_r tr - 1)))    (local; cross-core add of partial sums)
  td1 = mean_r sqrt(relu(-max_f nsq) + eps)       (cross-core max)
  td2 = mean_f sqrt(relu(-max_r nsq) + eps)       (local; cross-core add)
  out = rd1 + rd2 + pi * (td1 + td2)

Sharding: nF (4096) split across 8 cores, 512 fake poses per core; nR full.
Matmuls run in bf16 hi/lo split form (3 cross products, error ~1e-5 relative).
arccos(x) = 2*atan(sqrt((1-x)/(1+x))), with atan(t>1) = pi/2 - atan(1/t).
"""

import os
import sys

import numpy as np

for _p in ("/opt/trn_rl_repo",):
    if _p not in sys.path and os.path.isdir(_p):
        sys.path.insert(0, _p)

import ml_dtypes  # noqa: E402

import concourse.bacc as bacc  # noqa: E402
import concourse.mybir as mybir  # noqa: E402
from concourse import tile  # noqa: E402
from concourse.bass_utils import run_bass_kernel_spmd  # noqa: E402

NR = 8192
NF = 4096
NCORES = 8
NFL = NF // NCORES          # 512 fake poses per core
RB = NR // 128              # 64 r-blocks of 128
K_ROT = 27                  # 9 * 3 split terms (hh, hl, lh)
K_SQ = 15                   # 5 * 3
EVAC_DT = mybir.dt.float16
NEG_BIG = -60000.0
EPS_CH = 1e-5
EPS_ACOS = 1e-6
TR_LO = -1.0 + 2.0 * EPS_ACOS   # clip on tr equivalent to clip on cos=(tr-1)/2
TR_HI = 3.0 - 2.0 * EPS_ACOS
W = RB + 4                  # packed work width: 64 r-side cols + 4 f-side cols

_F32 = mybir.dt.float32
_BF16 = mybir.dt.bfloat16

_CACHE = {}


def _bf16_split(x):
    """x (fp32) -> (hi, lo) bf16 with hi + lo ~= x."""
    bf = ml_dtypes.bfloat16
    hi = x.astype(bf)
    lo = (x - hi.astype(np.float32)).astype(bf)
    return hi, lo


def _split_stack_left(xT):
    """Left operand stack [hi; hi; lo] for products hh + hl + lh."""
    hi, lo = _bf16_split(xT)
    return np.concatenate([hi, hi, lo], axis=0)


def _split_stack_right(xT):
    """Right operand stack [hi; lo; hi]."""
    hi, lo = _bf16_split(xT)
    return np.concatenate([hi, lo, hi], axis=0)


def _build_nc():
    nc = bacc.Bacc("TRN2", num_devices=NCORES, debug=False)
    mm = mybir.AluOpType

    lhsT_rot_d = nc.dram_tensor("lhsT_rot", [K_ROT, NR], _BF16, kind="ExternalInput")
    lhsT_sq_d = nc.dram_tensor("lhsT_sq", [K_SQ, NR], _BF16, kind="ExternalInput")
    rhs_rot_d = nc.dram_tensor("rhs_rot", [K_ROT, NFL], _BF16, kind="ExternalInput")
    rhs_sq_d = nc.dram_tensor("rhs_sq", [K_SQ, NFL], _BF16, kind="ExternalInput")
    ident_d = nc.dram_tensor("ident", [128, 128], EVAC_DT, kind="ExternalInput")
    out_d = nc.dram_tensor("out", [1, 1], _F32, kind="ExternalOutput")

    groups = [list(range(NCORES))]

    with tile.TileContext(nc) as tc:
        with (
            tc.tile_pool(name="const", bufs=1) as cpool,
            tc.tile_pool(name="mm", bufs=2, space="PSUM") as mmpool,
            tc.tile_pool(name="fin", bufs=1, space="PSUM") as finpool,
            tc.tile_pool(name="pair", bufs=4) as pairpool,
            tc.tile_pool(name="small", bufs=1) as spool,
            tc.tile_pool(name="dram", bufs=1, space="DRAM") as dpool,
        ):
            lhsT_rot = cpool.tile([K_ROT, NR], _BF16)
            lhsT_sq = cpool.tile([K_SQ, NR], _BF16)
            rhs_rot = cpool.tile([K_ROT, NFL], _BF16)
            rhs_sq = cpool.tile([K_SQ, NFL], _BF16)
            ident = cpool.tile([128, 128], EVAC_DT)
            nc.sync.dma_start(lhsT_rot[:], lhsT_rot_d[:])
            nc.sync.dma_start(lhsT_sq[:], lhsT_sq_d[:])
            nc.sync.dma_start(rhs_rot[:], rhs_rot_d[:])
            nc.sync.dma_start(rhs_sq[:], rhs_sq_d[:])
            nc.sync.dma_start(ident[:], ident_d[:])

            # Running per-f max over r-blocks; cols 0:512 rot tr, 512:1024 neg sq.
            acc = cpool.tile([128, 2 * NFL], EVAC_DT)
            nc.vector.memset(acc[:], NEG_BIG)
            # Per-r max over local f: [p, rb, 0] rot / [p, rb, 1] negsq, r = rb*128+p.
            # fp16 + innermost step-1 output keeps the DVE reduce in packed mode.
            maxf = cpool.tile([128, RB, 2], EVAC_DT)

            # ---- main loop over r-blocks ----
            for rb in range(RB):
                ps = mmpool.tile([128, 2 * NFL], _F32)
                cols = slice(rb * 128, (rb + 1) * 128)
                nc.tensor.matmul(
                    ps[:, 0:NFL], lhsT_rot[:, cols], rhs_rot[:], start=True, stop=True
                )
                nc.tensor.matmul(
                    ps[:, NFL : 2 * NFL], lhsT_sq[:, cols], rhs_sq[:],
                    start=True, stop=True,
                )
                pair = pairpool.tile([128, 2 * NFL], EVAC_DT)
                nc.scalar.copy(pair[:], ps[:])
                # Per-r max over the 512 local f: tensor_reduce has no packed
                # DVE modes (1x), so a TT max-tree (2x packed) + small reduce
                # is cheaper: 327+194+327 vs 1131 ns.
                pair3 = pair[:].rearrange("p (t n) -> p t n", t=2)
                t1 = pairpool.tile([128, 2, 256], EVAC_DT)
                nc.vector.tensor_tensor(
                    t1[:], pair3[:, :, 0:256], pair3[:, :, 256:512], op=mm.max
                )
                t2 = pairpool.tile([128, 2, 128], EVAC_DT)
                nc.vector.tensor_tensor(
                    t2[:], t1[:, :, 0:128], t1[:, :, 128:256], op=mm.max
                )
                nc.vector.tensor_reduce(
                    maxf[:, rb, :], t2[:], axis=mybir.AxisListType.X, op=mm.max
                )
                nc.vector.tensor_tensor(acc[:], pair[:], acc[:], op=mm.max)

            # ---- per-f stats: transpose acc, reduce over the 128 r-partials ----
            ft = finpool.tile([128, 1024], EVAC_DT)
            ff = spool.tile([128, 2, 4], _F32)
            for t in range(8):
                nc.tensor.transpose(
                    ft[:, t * 128 : (t + 1) * 128],
                    acc[:, t * 128 : (t + 1) * 128],
                    ident[:],
                )
                nc.vector.tensor_reduce(
                    ff[:, t // 4, t % 4 : t % 4 + 1],
                    ft[:, t * 128 : (t + 1) * 128],
                    axis=mybir.AxisListType.X,
                    op=mm.max,
                )

            # ---- collective 1: elementwise max of per-r stats across cores ----
            # Ship the stats in their interleaved SBUF layout (contiguous DMA);
            # AllReduce is elementwise so the layout is irrelevant cross-core.
            cc1_in = dpool.tile([128, RB * 2], EVAC_DT)
            cc1_out = dpool.tile([128, RB * 2], EVAC_DT)
            nc.sync.dma_start(
                cc1_in[:], maxf[:].rearrange("p a b -> p (a b)")
            )
            nc.gpsimd.collective_compute(
                "AllReduce",
                mm.max,
                replica_groups=groups,
                ins=[cc1_in[:].opt()],
                outs=[cc1_out[:].opt()],
            )
            gstat = spool.tile([128, RB, 2], EVAC_DT)
            nc.sync.dma_start(
                gstat[:].rearrange("p a b -> p (a b)"), cc1_out[:]
            )

            # ---- pack rot / negsq sources: cols 0:64 r-side, 64:68 f-side ----
            rot_src = spool.tile([128, W], _F32)
            neg_src = spool.tile([128, W], _F32)
            nc.vector.tensor_copy(rot_src[:, 0:RB], gstat[:, :, 0])
            nc.vector.tensor_copy(rot_src[:, RB:W], ff[:, 0, :])
            nc.vector.tensor_copy(neg_src[:, 0:RB], gstat[:, :, 1])
            nc.vector.tensor_copy(neg_src[:, RB:W], ff[:, 1, :])

            # ---- arccos pipeline on rot_src ----
            xc = spool.tile([128, W], _F32)
            num = spool.tile([128, W], _F32)
            den = spool.tile([128, W], _F32)
            ratio = spool.tile([128, W], _F32)
            root = spool.tile([128, W], _F32)
            rroot = spool.tile([128, W], _F32)
            mt = spool.tile([128, W], _F32)
            at = spool.tile([128, W], _F32)
            flag = spool.tile([128, W], _F32)
            sgn = spool.tile([128, W], _F32)
            ang2 = spool.tile([128, W], _F32)
            angle = spool.tile([128, W], _F32)
            zt = spool.tile([128, W], _F32)
            st = spool.tile([128, W], _F32)

            eps_ap = spool.tile([128, 1], _F32)
            nc.vector.memset(eps_ap[:], EPS_CH)

            nc.vector.tensor_scalar(xc[:], rot_src[:], TR_HI, TR_LO, mm.min, mm.max)
            nc.vector.tensor_scalar(num[:], xc[:], -0.5, 1.5, mm.mult, mm.add)
            nc.vector.tensor_scalar(den[:], xc[:], 0.5, 0.5, mm.mult, mm.add)
            nc.vector.reciprocal(den[:], den[:])
            nc.vector.tensor_tensor(ratio[:], num[:], den[:], op=mm.mult)
            # clamp of negsq for the sqrt path
            nc.vector.tensor_scalar_min(zt[:], neg_src[:], 0.0)

            # ACT phase 1: sqrt set (2 instructions)
            nc.scalar.activation(
                root[:], ratio[:], mybir.ActivationFunctionType.Sqrt
            )
            nc.scalar.activation(
                st[:], zt[:], mybir.ActivationFunctionType.Sqrt,
                bias=eps_ap[:], scale=-1.0,
            )

            nc.vector.reciprocal(rroot[:], root[:])
            nc.vector.tensor_tensor(mt[:], root[:], rroot[:], op=mm.min)

            # ACT phase 2: arctan set (1 instruction)
            nc.scalar.activation(at[:], mt[:], mybir.ActivationFunctionType.Arctan)

            # angle = 2*atan(root) = 2*at if root<=1 else pi - 2*at
            nc.vector.tensor_scalar(flag[:], root[:], 1.0, None, mm.is_gt)
            nc.vector.tensor_scalar(sgn[:], flag[:], -4.0, 2.0, mm.mult, mm.add)
            nc.vector.tensor_tensor(ang2[:], at[:], sgn[:], op=mm.mult)
            nc.vector.scalar_tensor_tensor(
                angle[:], flag[:], float(np.pi), ang2[:], op0=mm.mult, op1=mm.add
            )

            # ---- reduce to the four sums ----
            sums4 = spool.tile([128, 4], _F32)
            nc.vector.tensor_reduce(
                sums4[:, 0:1], angle[:, RB:W], axis=mybir.AxisListType.X, op=mm.add
            )  # rd1 partial (local f)
            nc.vector.tensor_reduce(
                sums4[:, 1:2], st[:, RB:W], axis=mybir.AxisListType.X, op=mm.add
            )  # td2 partial (local f)
            nc.vector.tensor_reduce(
                sums4[:, 2:3], angle[:, 0:RB], axis=mybir.AxisListType.X, op=mm.add
            )  # rd2 (global r)
            nc.vector.tensor_reduce(
                sums4[:, 3:4], st[:, 0:RB], axis=mybir.AxisListType.X, op=mm.add
            )  # td1 (global r)

            ones = spool.tile([128, 1], _F32)
            nc.vector.memset(ones[:], 1.0)
            s4 = finpool.tile([1, 4], _F32)
            nc.tensor.matmul(s4[:], ones[:], sums4[:], start=True, stop=True)

            # ---- collective 2: add the two f-side partial sums across cores ----
            z64 = spool.tile([1, 64], _F32)
            nc.vector.memset(z64[:], 0.0)
            nc.scalar.copy(z64[:, 0:2], s4[:, 0:2])
            cc2_in = dpool.tile([1, 64], _F32)
            cc2_out = dpool.tile([1, 64], _F32)
            nc.sync.dma_start(cc2_in[:], z64[:])
            nc.gpsimd.collective_compute(
                "AllReduce",
                mm.add,
                replica_groups=groups,
                ins=[cc2_in[:].opt()],
                outs=[cc2_out[:].opt()],
            )
            g2 = spool.tile([1, 2], _F32)
            nc.sync.dma_start(g2[:], cc2_out[:, 0:2])

            # ---- final combine ----
            t1 = spool.tile([1, 2], _F32)
            t2 = spool.tile([1, 2], _F32)
            ab = spool.tile([1, 2], _F32)
            res = spool.tile([1, 1], _F32)
            nc.vector.tensor_scalar_mul(t1[:], g2[:], 1.0 / NF)
            nc.vector.tensor_scalar_mul(t2[:], s4[:, 2:4], 1.0 / NR)
            nc.vector.tensor_tensor(ab[:], t1[:], t2[:], op=mm.add)
            nc.vector.scalar_tensor_tensor(
                res[:], ab[:, 1:2], float(np.pi), ab[:, 0:1],
                op0=mm.mult, op1=mm.add,
            )
            nc.sync.dma_start(out_d[:], res[:])

    nc.compile()
    return nc


def _prep_inputs(t_fake, R_fake, r_buffer, t_buffer):
    """Host-side shard + bf16 split. Returns per-core input maps."""
    t_fake = np.asarray(t_fake, dtype=np.float32)
    R_fake = np.asarray(R_fake, dtype=np.float32)
    r_buffer = np.asarray(r_buffer, dtype=np.float32)
    tb = np.asarray(t_buffer, dtype=np.float32)[0]          # [NR, 3]

    rotT = r_buffer.reshape(NR, 9).T                         # [9, NR]
    a_r = np.sum(tb * tb, axis=1)                            # [NR]
    # negated augmented vector: nsq = u_neg . v  with v = [1, |tf|^2, tf]
    u_negT = np.concatenate(
        [-a_r[None, :], -np.ones((1, NR), np.float32), 2.0 * tb.T], axis=0
    )                                                        # [5, NR]
    lhsT_rot = _split_stack_left(rotT)                       # [27, NR]
    lhsT_sq = _split_stack_left(u_negT)                      # [15, NR]

    ident = np.eye(128, dtype=np.float16)

    in_maps = []
    for c in range(NCORES):
        sl = slice(c * NFL, (c + 1) * NFL)
        RfT = R_fake.reshape(NF, 9)[sl].T                    # [9, NFL]
        tf = t_fake[sl]                                      # [NFL, 3]
        b_f = np.sum(tf * tf, axis=1)
        vT = np.concatenate(
            [np.ones((1, NFL), np.float32), b_f[None, :], tf.T], axis=0
        )                                                    # [5, NFL]
        in_maps.append(
            {
                "lhsT_rot": lhsT_rot,
                "lhsT_sq": lhsT_sq,
                "rhs_rot": _split_stack_right(RfT),          # [27, NFL]
                "rhs_sq": _split_stack_right(vT),            # [15, NFL]
                "ident": ident,
            }
        )
    return in_maps


LAST_RESULTS = None


def kernel(t_fake, R_fake, r_buffer, t_buffer):
    global LAST_RESULTS
    if "nc" not in _CACHE:
        _CACHE["nc"] = _build_nc()
    nc = _CACHE["nc"]
    in_maps = _prep_inputs(t_fake, R_fake, r_buffer, t_buffer)
    res = run_bass_kernel_spmd(nc, in_maps, list(range(NCORES)))
    LAST_RESULTS = res
    return np.float32(res.results[0]["out"][0, 0])
